# revision 16
# baseline (speedup 1.0000x reference)
"""Trainium2 Bass kernel for the HPNET loss (confidence + depth + rotation).

Contract: kernel(**inputs) takes the FULL unsharded inputs and returns the
full output (a tuple of three f32 scalars), distributing work across 8
NeuronCores internally.

Sharding (hardcoded): data-parallel over 8 cores.
  - confidence/confidence_gt/weight: batch dim 256 -> 32 batches per core,
    flattened per core to [128, 16384]. Streamed tensors are downcast on
    host (the 2e-2 loss tolerance leaves ample room; measured ~3e-4):
      a (confidence):  fp16
      b (confidence_gt): accum region as NEGATED fp8e4m3, tail fp16
      w (weight): fp8e4m3 (the weighted-accumulate STT runs in 1x mode
      regardless of dtype, so fp8 w is free compute-wise)
  - depth_and_rotation/ann_values/ann_flags: ROI dim 8192 -> 1024 per core,
    fused to one [128, 88] f32 tensor (dr | ann | mask).

d = a - b formation for the first 12288 columns happens INSIDE the DMA
engine: the gpsimd (SWDGE) queue accumulates the negated-fp8 b piece
into a's fp16 SBUF tile (CCE inline add, which also upconverts), so the
DVE never runs a subtract for 75% of the stream. Accum transfers are
limited to 2048 elems/partition (larger SWDGE accum descriptors fault).
Tail chunks use a regular fp16 DVE subtract; the last two also square on
the DVE so the post-stream tail has no cross-engine hops.

Rotation loss via the quaternion identity (no quat2mat matrices):
  tr(M(q)^T M(p)) = 4<q,p>^2 - |q|^2 |p|^2, and M(p) @ RY = M(p x r_y)
  with r_y the y-axis half-turn quaternion, so the second norm is the
  same formula with a signed component permutation of q_dr:
    ||G - P||^2      = 3 + 3 sA^2 + 2 sA - 8 <q_dr, q_ann>^2  / sD
    ||G - P RY||^2   = 3 + 3 sA^2 + 2 sA - 8 <q_dr', q_ann>^2 / sD
  min(n1, n2) = sqrt(base - 8 max(dot1, dot2)^2 / sD).

Per-core partial sums [128, n_chunks + 2] are reduced on host.
"""

import numpy as np

_NCORES = 8
_B = 256
_HW = 256 * 256
_N = 8192
_PB = _B // _NCORES            # batches per core
_F = _PB * _HW // 128          # 16384 free elems per partition
_CHUNKS = (2048, 2048, 2048, 2048, 2048, 2048, 2048, 1024, 512, 512)
_NBIG = 6                      # chunks with DMA-accum b (negated fp8)
_FBIG = sum(_CHUNKS[:_NBIG])   # 12288
assert sum(_CHUNKS) == _F
_NCH = len(_CHUNKS)
_NVO = 2                       # trailing chunks computed vector-only
_R = _N // _NCORES // 128      # 8 ROIs per partition
_OUTC = _NCH + 2

_CACHE = {}


def _emit_roi(nc, roi, f32, rt, accs):
    import concourse.mybir as mybir
    Alu = mybir.AluOpType
    Act = mybir.ActivationFunctionType
    AxX = mybir.AxisListType.X

    dr3 = rt[:, 0:_R * 5].rearrange("p (r c) -> p r c", c=5)  # [128, R, 5]
    an3 = rt[:, _R * 5:_R * 10].rearrange("p (r c) -> p r c", c=5)
    mt = rt[:, _R * 10:_R * 11]                    # [128, R]
    qd = dr3[:, :, 1:5]                            # [128, R, 4]
    qa = an3[:, :, 1:5]

    # depth loss partials
    dd = roi.tile([128, _R], f32, tag="dd", name="dd")
    nc.vector.tensor_sub(dd[:], dr3[:, :, 0], an3[:, :, 0])
    dd2 = roi.tile([128, _R], f32, tag="dd2", name="dd2")
    nc.scalar.activation(dd2[:], dd[:], Act.Square)
    dscr = roi.tile([128, _R], f32, tag="dscr", name="dscr")
    nc.vector.scalar_tensor_tensor(
        out=dscr[:], in0=dd2[:], scalar=1.0, in1=mt[:],
        op0=Alu.mult, op1=Alu.mult,
        accum_out=accs[:, _NCH:_NCH + 1])

    # q_dr' = (q2, q3, -q0, -q1): <q_dr', q_ann> = <q_dr, q_ann x r_y>
    qd2 = roi.tile([128, _R, 4], f32, tag="qd2", name="qd2")
    nc.vector.tensor_copy(qd2[:, :, 0:2], qd[:, :, 2:4])
    nc.vector.tensor_scalar_mul(qd2[:, :, 2:4], qd[:, :, 0:2], -1.0)

    # stacked products -> one reduce for sD, sA, dot1, dot2
    prod = roi.tile([128, 4, _R, 4], f32, tag="prod", name="prod")
    nc.vector.tensor_mul(prod[:, 0], qd, qd)
    nc.vector.tensor_mul(prod[:, 1], qa, qa)
    nc.vector.tensor_mul(prod[:, 2], qd, qa)
    nc.vector.tensor_mul(prod[:, 3], qd2[:], qa)
    red = roi.tile([128, 4, _R], f32, tag="red", name="red")
    nc.vector.tensor_reduce(out=red[:], in_=prod[:], axis=AxX, op=Alu.add)

    rinv = roi.tile([128, _R], f32, tag="rinv", name="rinv")
    nc.vector.reciprocal(rinv[:], red[:, 0, :])
    dsq = roi.tile([128, 2, _R], f32, tag="dsq", name="dsq")
    nc.vector.tensor_mul(dsq[:], red[:, 2:4, :], red[:, 2:4, :])
    kmax = roi.tile([128, _R], f32, tag="kmax", name="kmax")
    nc.vector.tensor_tensor(kmax[:], dsq[:, 0, :], dsq[:, 1, :], op=Alu.max)
    k = roi.tile([128, _R], f32, tag="k", name="k")
    nc.vector.tensor_mul(k[:], kmax[:], rinv[:])

    # base' = 1.5 sA^2 + sA ; nmin^2 = 2*(base' - 4k) + 3
    sa2 = roi.tile([128, _R], f32, tag="sa2", name="sa2")
    nc.scalar.activation(sa2[:], red[:, 1, :], Act.Square)
    basep = roi.tile([128, _R], f32, tag="basep", name="basep")
    nc.vector.scalar_tensor_tensor(
        out=basep[:], in0=sa2[:], scalar=1.5, in1=red[:, 1, :],
        op0=Alu.mult, op1=Alu.add)
    mp = roi.tile([128, _R], f32, tag="mp", name="mp")
    nc.vector.scalar_tensor_tensor(
        out=mp[:], in0=k[:], scalar=-4.0, in1=basep[:],
        op0=Alu.mult, op1=Alu.add)
    b3 = roi.tile([128, 1], f32, tag="b3", name="b3")
    nc.gpsimd.memset(b3[:], 3.0)
    n = roi.tile([128, _R], f32, tag="n", name="n")
    nc.scalar.activation(n[:], mp[:], Act.Sqrt, bias=b3[:], scale=2.0)
    rscr = roi.tile([128, _R], f32, tag="rscr", name="rscr")
    nc.vector.scalar_tensor_tensor(
        out=rscr[:], in0=n[:], scalar=1.0, in1=mt[:],
        op0=Alu.mult, op1=Alu.mult,
        accum_out=accs[:, _NCH + 1:_NCH + 2])


def build_nc():
    import concourse.bass as cbass
    import concourse.bacc as bacc
    import concourse.mybir as mybir
    import concourse.tile as tile

    base = cbass.get_walrus_max_sem_num()
    cbass.get_kernel_semaphore_range = lambda: range(base, 200)

    f32 = mybir.dt.float32
    f16 = mybir.dt.float16
    f8 = mybir.dt.float8e4
    Alu = mybir.AluOpType
    Act = mybir.ActivationFunctionType

    nc = bacc.Bacc("TRN2", target_bir_lowering=False, debug=False,
                   num_devices=_NCORES)

    a = nc.dram_tensor("a", [128, _F], f16, kind="ExternalInput")
    bn8 = nc.dram_tensor("bn8", [128, _FBIG], f8, kind="ExternalInput")
    b16 = nc.dram_tensor("b16", [128, _F - _FBIG], f16,
                         kind="ExternalInput")
    w = nc.dram_tensor("w", [128, _F], f8, kind="ExternalInput")
    rio = nc.dram_tensor("rio", [128, _R * 11], f32, kind="ExternalInput")
    out = nc.dram_tensor("out", [128, _OUTC], f32, kind="ExternalOutput")

    with tile.TileContext(nc) as tc:
        with tc.tile_pool(name="io", bufs=4) as io, \
                tc.tile_pool(name="iow", bufs=_NCH) as iow, \
                tc.tile_pool(name="wk", bufs=3) as wk, \
                tc.tile_pool(name="roi", bufs=1) as roi:

            accs = roi.tile([128, _OUTC], f32, tag="accs", name="accs")

            tiles = []
            off = 0
            for i, ch in enumerate(_CHUNKS):
                at = io.tile([128, ch], f16, tag="at", name="at")
                bt = (io.tile([128, ch], f16, tag="bt", name="bt")
                      if i >= _NBIG else None)
                wt = iow.tile([128, ch], f8, tag="wt", name="wt")
                tiles.append((at, bt, wt, slice(off, off + ch)))
                off += ch

            qbytes = {"sync": 0, "scalar": 0}

            def hwdge(dst, src, nbytes):
                # keep both HWDGE queues draining in step
                if qbytes["sync"] <= qbytes["scalar"]:
                    nc.sync.dma_start(out=dst, in_=src)
                    qbytes["sync"] += nbytes
                else:
                    nc.scalar.dma_start(out=dst, in_=src)
                    qbytes["scalar"] += nbytes

            def load_chunk(i):
                at, bt, wt, sl = tiles[i]
                ch = _CHUNKS[i]
                hwdge(at[:], a[:, sl], 2 * ch)
                if i < _NBIG:
                    # negated fp8 b accumulated into a's fp16 tile by the
                    # DMA engine's inline adder -> at becomes d = a - b
                    nc.gpsimd.dma_start(out=at[:], in_=bn8[:, sl],
                                        accum_op=Alu.add)
                else:
                    hwdge(bt[:],
                          b16[:, sl.start - _FBIG:sl.stop - _FBIG], 2 * ch)
                hwdge(wt[:], w[:, sl], ch)

            # ROI input first on sync (tiny, lands during engine init)
            rt = roi.tile([128, _R * 11], f32, tag="rt", name="rt")
            nc.sync.dma_start(out=rt[:], in_=rio[:])
            load_chunk(0)

            # Preload the Sqrt act table during init
            zz = nc.const_aps.tensor(0.0, (128, 1))
            dummy = roi.tile([128, 1], f32, tag="dummy", name="dummy")
            nc.scalar.activation(dummy[:], zz, Act.Sqrt)

            _emit_roi(nc, roi, f32, rt, accs)

            ds = [None] * _NCH

            def emit_sq(i):
                at, bt, _, _ = tiles[i]
                if i < _NBIG:
                    d = at          # DMA already formed a - b here
                else:
                    d = wk.tile([128, _CHUNKS[i]], f16, tag="d", name="d")
                    nc.vector.tensor_sub(d[:], at[:], bt[:])
                if i >= _NCH - _NVO:
                    # tail chunks: square on the DVE, no cross-engine hop
                    nc.vector.tensor_mul(d[:], d[:], d[:])
                else:
                    nc.scalar.activation(d[:], d[:], Act.Square)
                ds[i] = d

            def emit_stt(i):
                _, _, wt, _ = tiles[i]
                d = ds[i]
                nc.vector.scalar_tensor_tensor(
                    out=d[:], in0=d[:], scalar=1.0, in1=wt[:],
                    op0=Alu.mult, op1=Alu.mult,
                    accum_out=accs[:, i:i + 1])

            for i in range(_NCH):
                if i + 1 < _NCH:
                    load_chunk(i + 1)
                emit_sq(i)
                if i > 0:
                    emit_stt(i - 1)
            emit_stt(_NCH - 1)

            nc.sync.dma_start(out=out[:], in_=accs[:])

    nc.compile()
    return nc


def _get_nc():
    if "nc" not in _CACHE:
        _CACHE["nc"] = build_nc()
    return _CACHE["nc"]


def make_in_maps(confidence, confidence_gt, weight, depth_and_rotation,
                 ann_values, ann_flags):
    import ml_dtypes
    f8 = ml_dtypes.float8_e4m3
    a = np.ascontiguousarray(confidence, dtype=np.float16).reshape(
        _NCORES, 128, _F)
    bf = np.ascontiguousarray(confidence_gt, dtype=np.float32).reshape(
        _NCORES, 128, _F)
    bn8 = np.ascontiguousarray((-bf[:, :, :_FBIG])).astype(f8)
    b16 = np.ascontiguousarray(bf[:, :, _FBIG:]).astype(np.float16)
    w = np.ascontiguousarray(weight, dtype=np.float32).reshape(
        _NCORES, 128, _F).astype(f8)
    dr = np.ascontiguousarray(depth_and_rotation, dtype=np.float32).reshape(
        _NCORES, 128, _R * 5)
    an = np.ascontiguousarray(ann_values, dtype=np.float32).reshape(
        _NCORES, 128, _R * 5)
    mk = np.ascontiguousarray(ann_flags).astype(np.float32).reshape(
        _NCORES, 128, _R)
    rio = np.concatenate([dr, an, mk], axis=2)     # [cores, 128, R*11]
    return [dict(a=a[c], bn8=bn8[c], b16=b16[c], w=w[c], rio=rio[c])
            for c in range(_NCORES)]


def reduce_outs(outs):
    """outs: list of per-core {'out': [128, _OUTC]} -> (conf, depth, rot)."""
    P = np.stack([o["out"] for o in outs]).astype(np.float64)
    conf = P[:, :, :_NCH].sum() / float(_HW)
    dep = P[:, :, _NCH].sum() / float(_N)
    rot = P[:, :, _NCH + 1].sum() / float(_N)
    return (np.float32(conf), np.float32(dep), np.float32(rot))


def kernel(confidence, confidence_gt, weight, depth_and_rotation,
           ann_values, ann_flags):
    from concourse.bass_utils import run_bass_kernel_spmd
    nc = _get_nc()
    in_maps = make_in_maps(confidence, confidence_gt, weight,
                           depth_and_rotation, ann_values, ann_flags)
    res = run_bass_kernel_spmd(nc, in_maps, core_ids=list(range(_NCORES)))
    return reduce_outs(res.results)


# revision 18
# speedup vs baseline: 1.0370x; 1.0370x over previous
"""Trainium2 Bass kernel for the HPNET loss (confidence + depth + rotation).

Contract: kernel(**inputs) takes the FULL unsharded inputs and returns the
full output (a tuple of three f32 scalars), distributing work across 8
NeuronCores internally.

Sharding (hardcoded): data-parallel over 8 cores.
  - confidence/confidence_gt/weight: batch dim 256 -> 32 batches per core,
    flattened per core to [128, 16384], downcast to fp16 on host (the
    streamed tensors dominate HBM traffic; fp16 halves it and the loss
    tolerates it: measured rel err ~1e-5).
  - depth_and_rotation/ann_values/ann_flags: ROI dim 8192 -> 1024 per core,
    laid out as [128, 8 ROIs * 5] f32 (flags as f32 mask [128, 8]).

Rotation loss via the quaternion identity (no quat2mat matrices):
  tr(M(q)^T M(p)) = 4<q,p>^2 - |q|^2 |p|^2  for the (unnormalized)
  quaternion-to-matrix map M, and M(p) @ RY = M(p x r_y) where r_y is the
  y-axis half-turn quaternion, so p' = p x r_y is just a signed component
  permutation. With G = M(q_dr / |q_dr|) and P = M(q_ann):
    ||G - P||_F^2      = 3 + 3 sA^2 + 2 sA - 8 <q_dr, q_ann>^2  / sD
    ||G - P RY||_F^2   = 3 + 3 sA^2 + 2 sA - 8 <q_dr, q_ann'>^2 / sD
  (sD = |q_dr|^2, sA = |q_ann|^2), and min(n1, n2) = sqrt(base - 8*max/sD).

DMA queues: a -> sync (HWDGE), b -> gpsimd (SWDGE), w -> scalar (HWDGE,
all chunks pre-issued before the Act squares so DMA issue never queues
behind compute). Per-core partial sums [128, n_chunks + 2] are reduced
on host.
"""

import numpy as np

_NCORES = 8
_B = 256
_HW = 256 * 256
_N = 8192
_PB = _B // _NCORES            # batches per core
_F = _PB * _HW // 128          # 16384 free elems per partition
_CHUNKS = (4096, 4096, 4096, 2048, 1024, 512, 512)
assert sum(_CHUNKS) == _F
_NCH = len(_CHUNKS)
_R = _N // _NCORES // 128      # 8 ROIs per partition
_OUTC = _NCH + 2

_CACHE = {}


def _emit_roi(nc, roi, f32, rt, accs):
    import concourse.mybir as mybir
    Alu = mybir.AluOpType
    Act = mybir.ActivationFunctionType
    AxX = mybir.AxisListType.X

    dr3 = rt[:, 0:_R * 5].rearrange("p (r c) -> p r c", c=5)  # [128, R, 5]
    an3 = rt[:, _R * 5:_R * 10].rearrange("p (r c) -> p r c", c=5)
    mt = rt[:, _R * 10:_R * 11]                    # [128, R]
    qd = dr3[:, :, 1:5]                            # [128, R, 4]
    qa = an3[:, :, 1:5]

    # depth loss partials
    dd = roi.tile([128, _R], f32, tag="dd", name="dd")
    nc.vector.tensor_sub(dd[:], dr3[:, :, 0], an3[:, :, 0])
    dd2 = roi.tile([128, _R], f32, tag="dd2", name="dd2")
    nc.scalar.activation(dd2[:], dd[:], Act.Square)
    dscr = roi.tile([128, _R], f32, tag="dscr", name="dscr")
    nc.vector.scalar_tensor_tensor(
        out=dscr[:], in0=dd2[:], scalar=1.0, in1=mt[:],
        op0=Alu.mult, op1=Alu.mult,
        accum_out=accs[:, _NCH:_NCH + 1])

    # q_dr' = (q2, q3, -q0, -q1): <q_dr', q_ann> = <q_dr, q_ann x r_y>
    qd2 = roi.tile([128, _R, 4], f32, tag="qd2", name="qd2")
    nc.vector.tensor_copy(qd2[:, :, 0:2], qd[:, :, 2:4])
    nc.vector.tensor_scalar_mul(qd2[:, :, 2:4], qd[:, :, 0:2], -1.0)

    # stacked products -> one reduce for sD, sA, dot1, dot2
    prod = roi.tile([128, 4, _R, 4], f32, tag="prod", name="prod")
    nc.vector.tensor_mul(prod[:, 0], qd, qd)
    nc.vector.tensor_mul(prod[:, 1], qa, qa)
    nc.vector.tensor_mul(prod[:, 2], qd, qa)
    nc.vector.tensor_mul(prod[:, 3], qd2[:], qa)
    red = roi.tile([128, 4, _R], f32, tag="red", name="red")
    nc.vector.tensor_reduce(out=red[:], in_=prod[:], axis=AxX, op=Alu.add)

    rinv = roi.tile([128, _R], f32, tag="rinv", name="rinv")
    nc.vector.reciprocal(rinv[:], red[:, 0, :])
    dsq = roi.tile([128, 2, _R], f32, tag="dsq", name="dsq")
    nc.vector.tensor_mul(dsq[:], red[:, 2:4, :], red[:, 2:4, :])
    kmax = roi.tile([128, _R], f32, tag="kmax", name="kmax")
    nc.vector.tensor_tensor(kmax[:], dsq[:, 0, :], dsq[:, 1, :], op=Alu.max)
    k = roi.tile([128, _R], f32, tag="k", name="k")
    nc.vector.tensor_mul(k[:], kmax[:], rinv[:])

    # base' = 1.5 sA^2 + sA ; nmin^2 = 2*(base' - 4k) + 3
    sa2 = roi.tile([128, _R], f32, tag="sa2", name="sa2")
    nc.scalar.activation(sa2[:], red[:, 1, :], Act.Square)
    basep = roi.tile([128, _R], f32, tag="basep", name="basep")
    nc.vector.scalar_tensor_tensor(
        out=basep[:], in0=sa2[:], scalar=1.5, in1=red[:, 1, :],
        op0=Alu.mult, op1=Alu.add)
    mp = roi.tile([128, _R], f32, tag="mp", name="mp")
    nc.vector.scalar_tensor_tensor(
        out=mp[:], in0=k[:], scalar=-4.0, in1=basep[:],
        op0=Alu.mult, op1=Alu.add)
    b3 = roi.tile([128, 1], f32, tag="b3", name="b3")
    nc.gpsimd.memset(b3[:], 3.0)
    n = roi.tile([128, _R], f32, tag="n", name="n")
    nc.scalar.activation(n[:], mp[:], Act.Sqrt, bias=b3[:], scale=2.0)
    rscr = roi.tile([128, _R], f32, tag="rscr", name="rscr")
    nc.vector.scalar_tensor_tensor(
        out=rscr[:], in0=n[:], scalar=1.0, in1=mt[:],
        op0=Alu.mult, op1=Alu.mult,
        accum_out=accs[:, _NCH + 1:_NCH + 2])


def build_nc():
    import concourse.bass as cbass
    import concourse.bacc as bacc
    import concourse.mybir as mybir
    import concourse.tile as tile

    # Shrink the kernel semaphore file: the framework initializes and
    # resets every semaphore in the range at kernel begin/end (one
    # instruction each, split across engines), so a smaller file directly
    # shortens the fixed prologue/epilogue.
    base = cbass.get_walrus_max_sem_num()
    cbass.get_kernel_semaphore_range = lambda: range(base, 200)

    f32 = mybir.dt.float32
    f16 = mybir.dt.float16
    f8 = mybir.dt.float8e4
    Alu = mybir.AluOpType
    Act = mybir.ActivationFunctionType

    nc = bacc.Bacc("TRN2", target_bir_lowering=False, debug=False,
                   num_devices=_NCORES)

    a = nc.dram_tensor("a", [128, _F], f16, kind="ExternalInput")
    b = nc.dram_tensor("b", [128, _F], f16, kind="ExternalInput")
    w = nc.dram_tensor("w", [128, _F], f8, kind="ExternalInput")
    rio = nc.dram_tensor("rio", [128, _R * 11], f32, kind="ExternalInput")
    out = nc.dram_tensor("out", [128, _OUTC], f32, kind="ExternalOutput")

    with tile.TileContext(nc) as tc:
        with tc.tile_pool(name="io", bufs=3) as io, \
                tc.tile_pool(name="iow", bufs=_NCH) as iow, \
                tc.tile_pool(name="wk", bufs=4) as wk, \
                tc.tile_pool(name="roi", bufs=1) as roi:

            accs = roi.tile([128, _OUTC], f32, tag="accs", name="accs")

            tiles = []
            off = 0
            for i, ch in enumerate(_CHUNKS):
                at = io.tile([128, ch], f16, tag="at", name="at")
                bt = io.tile([128, ch], f16, tag="bt", name="bt")
                wt = iow.tile([128, ch], f8, tag="wt", name="wt")
                tiles.append((at, bt, wt, slice(off, off + ch)))
                off += ch

            # ROI input first on sync (tiny, lands during engine init),
            # then a0/b0 so the HBM stream starts immediately.
            rt = roi.tile([128, _R * 11], f32, tag="rt", name="rt")
            nc.sync.dma_start(out=rt[:], in_=rio[:])
            at, bt, _, sl = tiles[0]
            nc.sync.dma_start(out=at[:], in_=a[:, sl])
            nc.scalar.dma_start(out=bt[:], in_=b[:, sl])

            # Preload the Sqrt act table during init (its only real use is
            # the ROI chain; an inline load there would stall the stream).
            zz = nc.const_aps.tensor(0.0, (128, 1))
            dummy = roi.tile([128, 1], f32, tag="dummy", name="dummy")
            nc.scalar.activation(dummy[:], zz, Act.Sqrt)

            # ROI losses: short serial chain hidden under chunk-0 transfers
            _emit_roi(nc, roi, f32, rt, accs)

            # w chunks go to whichever HWDGE queue is lighter so both
            # queues drain in step (HBM, not a single queue, is the cap)
            qbytes = {"sync": 0, "scalar": 0}

            def load_w(i):
                _, _, wt, sl = tiles[i]
                ch = _CHUNKS[i]
                if qbytes["sync"] <= qbytes["scalar"]:
                    nc.sync.dma_start(out=wt[:], in_=w[:, sl])
                    qbytes["sync"] += ch
                else:
                    nc.scalar.dma_start(out=wt[:], in_=w[:, sl])
                    qbytes["scalar"] += ch

            load_w(0)
            ds = [None] * _NCH

            def emit_sub(i):
                at, bt, _, _ = tiles[i]
                d = wk.tile([128, _CHUNKS[i]], f16, tag="d", name="d")
                # two big subs run in Q7 software on the idle Pool engine
                seng = nc.gpsimd if i in (1, 2) else nc.vector
                seng.tensor_sub(d[:], at[:], bt[:])
                if i >= _NCH - 2:
                    # tail: square on the DVE, no cross-engine hop
                    nc.vector.tensor_mul(d[:], d[:], d[:])
                else:
                    nc.scalar.activation(d[:], d[:], Act.Square)
                ds[i] = d

            def emit_stt(i):
                _, _, wt, _ = tiles[i]
                d = ds[i]
                # the two big leading chunks' weighted accumulates run in
                # Q7 software on the otherwise-idle Pool engine; they have
                # the whole stream as slack
                nc.vector.scalar_tensor_tensor(
                    out=d[:], in0=d[:], scalar=1.0, in1=wt[:],
                    op0=Alu.mult, op1=Alu.mult,
                    accum_out=accs[:, i:i + 1])

            for i in range(_NCH):
                if i + 1 < _NCH:
                    at, bt, _, sl = tiles[i + 1]
                    nc.sync.dma_start(out=at[:], in_=a[:, sl])
                    nc.scalar.dma_start(out=bt[:], in_=b[:, sl])
                    load_w(i + 1)
                emit_sub(i)
                if i > 0:
                    emit_stt(i - 1)
            emit_stt(_NCH - 1)

            nc.sync.dma_start(out=out[:], in_=accs[:])

    nc.compile()
    return nc


def _get_nc():
    if "nc" not in _CACHE:
        _CACHE["nc"] = build_nc()
    return _CACHE["nc"]


def make_in_maps(confidence, confidence_gt, weight, depth_and_rotation,
                 ann_values, ann_flags):
    a = np.ascontiguousarray(confidence, dtype=np.float16).reshape(
        _NCORES, 128, _F)
    b = np.ascontiguousarray(confidence_gt, dtype=np.float16).reshape(
        _NCORES, 128, _F)
    import ml_dtypes
    w = np.ascontiguousarray(weight, dtype=np.float32).reshape(
        _NCORES, 128, _F).astype(ml_dtypes.float8_e4m3)
    dr = np.ascontiguousarray(depth_and_rotation, dtype=np.float32).reshape(
        _NCORES, 128, _R * 5)
    an = np.ascontiguousarray(ann_values, dtype=np.float32).reshape(
        _NCORES, 128, _R * 5)
    mk = np.ascontiguousarray(ann_flags).astype(np.float32).reshape(
        _NCORES, 128, _R)
    rio = np.concatenate([dr, an, mk], axis=2)     # [cores, 128, R*11]
    return [dict(a=a[c], b=b[c], w=w[c], rio=rio[c])
            for c in range(_NCORES)]


def reduce_outs(outs):
    """outs: list of per-core {'out': [128, _OUTC]} -> (conf, depth, rot)."""
    P = np.stack([o["out"] for o in outs]).astype(np.float64)
    conf = P[:, :, :_NCH].sum() / float(_HW)
    dep = P[:, :, _NCH].sum() / float(_N)
    rot = P[:, :, _NCH + 1].sum() / float(_N)
    return (np.float32(conf), np.float32(dep), np.float32(rot))


def kernel(confidence, confidence_gt, weight, depth_and_rotation,
           ann_values, ann_flags):
    from concourse.bass_utils import run_bass_kernel_spmd
    nc = _get_nc()
    in_maps = make_in_maps(confidence, confidence_gt, weight,
                           depth_and_rotation, ann_values, ann_flags)
    res = run_bass_kernel_spmd(nc, in_maps, core_ids=list(range(_NCORES)))
    return reduce_outs(res.results)


# revision 19
# speedup vs baseline: 1.1625x; 1.1211x over previous
"""Trainium2 Bass kernel for the HPNET loss (confidence + depth + rotation).

Contract: kernel(**inputs) takes the FULL unsharded inputs and returns the
full output (a tuple of three f32 scalars), distributing work across 8
NeuronCores internally.

Sharding (hardcoded): data-parallel over 8 cores.
  - confidence/confidence_gt/weight: batch dim 256 -> 32 batches per core,
    flattened per core to [128, 16384], downcast to fp16 on host (the
    streamed tensors dominate HBM traffic; fp16 halves it and the loss
    tolerates it: measured rel err ~1e-5).
  - depth_and_rotation/ann_values/ann_flags: ROI dim 8192 -> 1024 per core,
    laid out as [128, 8 ROIs * 5] f32 (flags as f32 mask [128, 8]).

Rotation loss via the quaternion identity (no quat2mat matrices):
  tr(M(q)^T M(p)) = 4<q,p>^2 - |q|^2 |p|^2  for the (unnormalized)
  quaternion-to-matrix map M, and M(p) @ RY = M(p x r_y) where r_y is the
  y-axis half-turn quaternion, so p' = p x r_y is just a signed component
  permutation. With G = M(q_dr / |q_dr|) and P = M(q_ann):
    ||G - P||_F^2      = 3 + 3 sA^2 + 2 sA - 8 <q_dr, q_ann>^2  / sD
    ||G - P RY||_F^2   = 3 + 3 sA^2 + 2 sA - 8 <q_dr, q_ann'>^2 / sD
  (sD = |q_dr|^2, sA = |q_ann|^2), and min(n1, n2) = sqrt(base - 8*max/sD).

DMA queues: a -> sync (HWDGE), b -> gpsimd (SWDGE), w -> scalar (HWDGE,
all chunks pre-issued before the Act squares so DMA issue never queues
behind compute). Per-core partial sums [128, n_chunks + 2] are reduced
on host.
"""

import numpy as np

_NCORES = 8
_B = 256
_HW = 256 * 256
_N = 8192
_PB = _B // _NCORES            # batches per core
_F = _PB * _HW // 128          # 16384 free elems per partition
_CHUNKS = (2048, 2048, 4096, 4096, 2048, 1024, 512, 512)
assert sum(_CHUNKS) == _F
_NCH = len(_CHUNKS)
_R = _N // _NCORES // 128      # 8 ROIs per partition
_OUTC = _NCH + 2

_CACHE = {}


def _emit_roi(nc, roi, f32, rt, accs):
    import concourse.mybir as mybir
    Alu = mybir.AluOpType
    Act = mybir.ActivationFunctionType
    AxX = mybir.AxisListType.X

    dr3 = rt[:, 0:_R * 5].rearrange("p (r c) -> p r c", c=5)  # [128, R, 5]
    an3 = rt[:, _R * 5:_R * 10].rearrange("p (r c) -> p r c", c=5)
    mt = rt[:, _R * 10:_R * 11]                    # [128, R]
    qd = dr3[:, :, 1:5]                            # [128, R, 4]
    qa = an3[:, :, 1:5]

    # depth loss partials
    dd = roi.tile([128, _R], f32, tag="dd", name="dd")
    nc.vector.tensor_sub(dd[:], dr3[:, :, 0], an3[:, :, 0])
    dd2 = roi.tile([128, _R], f32, tag="dd2", name="dd2")
    nc.scalar.activation(dd2[:], dd[:], Act.Square)
    dscr = roi.tile([128, _R], f32, tag="dscr", name="dscr")
    nc.vector.scalar_tensor_tensor(
        out=dscr[:], in0=dd2[:], scalar=1.0, in1=mt[:],
        op0=Alu.mult, op1=Alu.mult,
        accum_out=accs[:, _NCH:_NCH + 1])

    # q_dr' = (q2, q3, -q0, -q1): <q_dr', q_ann> = <q_dr, q_ann x r_y>
    qd2 = roi.tile([128, _R, 4], f32, tag="qd2", name="qd2")
    nc.vector.tensor_copy(qd2[:, :, 0:2], qd[:, :, 2:4])
    nc.vector.tensor_scalar_mul(qd2[:, :, 2:4], qd[:, :, 0:2], -1.0)

    # stacked products -> one reduce for sD, sA, dot1, dot2
    prod = roi.tile([128, 4, _R, 4], f32, tag="prod", name="prod")
    nc.vector.tensor_mul(prod[:, 0], qd, qd)
    nc.vector.tensor_mul(prod[:, 1], qa, qa)
    nc.vector.tensor_mul(prod[:, 2], qd, qa)
    nc.vector.tensor_mul(prod[:, 3], qd2[:], qa)
    red = roi.tile([128, 4, _R], f32, tag="red", name="red")
    nc.vector.tensor_reduce(out=red[:], in_=prod[:], axis=AxX, op=Alu.add)

    rinv = roi.tile([128, _R], f32, tag="rinv", name="rinv")
    nc.vector.reciprocal(rinv[:], red[:, 0, :])
    dsq = roi.tile([128, 2, _R], f32, tag="dsq", name="dsq")
    nc.vector.tensor_mul(dsq[:], red[:, 2:4, :], red[:, 2:4, :])
    kmax = roi.tile([128, _R], f32, tag="kmax", name="kmax")
    nc.vector.tensor_tensor(kmax[:], dsq[:, 0, :], dsq[:, 1, :], op=Alu.max)
    k = roi.tile([128, _R], f32, tag="k", name="k")
    nc.vector.tensor_mul(k[:], kmax[:], rinv[:])

    # base' = 1.5 sA^2 + sA ; nmin^2 = 2*(base' - 4k) + 3
    sa2 = roi.tile([128, _R], f32, tag="sa2", name="sa2")
    nc.scalar.activation(sa2[:], red[:, 1, :], Act.Square)
    basep = roi.tile([128, _R], f32, tag="basep", name="basep")
    nc.vector.scalar_tensor_tensor(
        out=basep[:], in0=sa2[:], scalar=1.5, in1=red[:, 1, :],
        op0=Alu.mult, op1=Alu.add)
    mp = roi.tile([128, _R], f32, tag="mp", name="mp")
    nc.vector.scalar_tensor_tensor(
        out=mp[:], in0=k[:], scalar=-4.0, in1=basep[:],
        op0=Alu.mult, op1=Alu.add)
    b3 = roi.tile([128, 1], f32, tag="b3", name="b3")
    nc.gpsimd.memset(b3[:], 3.0)
    n = roi.tile([128, _R], f32, tag="n", name="n")
    nc.scalar.activation(n[:], mp[:], Act.Sqrt, bias=b3[:], scale=2.0)
    rscr = roi.tile([128, _R], f32, tag="rscr", name="rscr")
    nc.vector.scalar_tensor_tensor(
        out=rscr[:], in0=n[:], scalar=1.0, in1=mt[:],
        op0=Alu.mult, op1=Alu.mult,
        accum_out=accs[:, _NCH + 1:_NCH + 2])


def build_nc():
    import concourse.bass as cbass
    import concourse.bacc as bacc
    import concourse.mybir as mybir
    import concourse.tile as tile

    # Shrink the kernel semaphore file: the framework initializes and
    # resets every semaphore in the range at kernel begin/end (one
    # instruction each, split across engines), so a smaller file directly
    # shortens the fixed prologue/epilogue.
    base = cbass.get_walrus_max_sem_num()
    cbass.get_kernel_semaphore_range = lambda: range(base, 200)

    f32 = mybir.dt.float32
    f16 = mybir.dt.float16
    f8 = mybir.dt.float8e4
    Alu = mybir.AluOpType
    Act = mybir.ActivationFunctionType

    nc = bacc.Bacc("TRN2", target_bir_lowering=False, debug=False,
                   num_devices=_NCORES)

    a = nc.dram_tensor("a", [128, _F], f16, kind="ExternalInput")
    b = nc.dram_tensor("b", [128, _F], f16, kind="ExternalInput")
    w = nc.dram_tensor("w", [128, _F], f8, kind="ExternalInput")
    rio = nc.dram_tensor("rio", [128, _R * 11], f32, kind="ExternalInput")
    out = nc.dram_tensor("out", [128, _OUTC], f32, kind="ExternalOutput")

    with tile.TileContext(nc) as tc:
        with tc.tile_pool(name="io", bufs=4) as io, \
                tc.tile_pool(name="iow", bufs=_NCH) as iow, \
                tc.tile_pool(name="wk", bufs=3) as wk, \
                tc.tile_pool(name="roi", bufs=1) as roi:

            accs = roi.tile([128, _OUTC], f32, tag="accs", name="accs")

            tiles = []
            off = 0
            for i, ch in enumerate(_CHUNKS):
                at = io.tile([128, ch], f16, tag="at", name="at")
                bt = io.tile([128, ch], f16, tag="bt", name="bt")
                wt = iow.tile([128, ch], f8, tag="wt", name="wt")
                tiles.append((at, bt, wt, slice(off, off + ch)))
                off += ch

            # ROI input first on sync (tiny, lands during engine init),
            # then a0/b0 so the HBM stream starts immediately.
            rt = roi.tile([128, _R * 11], f32, tag="rt", name="rt")
            nc.sync.dma_start(out=rt[:], in_=rio[:])
            at, bt, _, sl = tiles[0]
            nc.sync.dma_start(out=at[:], in_=a[:, sl])
            nc.scalar.dma_start(out=bt[:], in_=b[:, sl])

            # Preload the Sqrt act table during init (its only real use is
            # the ROI chain; an inline load there would stall the stream).
            zz = nc.const_aps.tensor(0.0, (128, 1))
            dummy = roi.tile([128, 1], f32, tag="dummy", name="dummy")
            nc.scalar.activation(dummy[:], zz, Act.Sqrt)

            # ROI losses: short serial chain hidden under chunk-0 transfers
            _emit_roi(nc, roi, f32, rt, accs)

            # w chunks go to whichever HWDGE queue is lighter so both
            # queues drain in step (HBM, not a single queue, is the cap)
            qbytes = {"sync": 0, "scalar": 0}

            def load_w(i):
                _, _, wt, sl = tiles[i]
                ch = _CHUNKS[i]
                if qbytes["sync"] <= qbytes["scalar"]:
                    nc.sync.dma_start(out=wt[:], in_=w[:, sl])
                    qbytes["sync"] += ch
                else:
                    nc.scalar.dma_start(out=wt[:], in_=w[:, sl])
                    qbytes["scalar"] += ch

            load_w(0)
            ds = [None] * _NCH

            def emit_sub(i):
                at, bt, _, _ = tiles[i]
                d = wk.tile([128, _CHUNKS[i]], f16, tag="d", name="d")
                nc.vector.tensor_sub(d[:], at[:], bt[:])
                if i >= _NCH - 2:
                    nc.vector.tensor_mul(d[:], d[:], d[:])
                else:
                    nc.scalar.activation(d[:], d[:], Act.Square)
                ds[i] = d

            def emit_stt(i):
                _, _, wt, _ = tiles[i]
                d = ds[i]
                nc.vector.scalar_tensor_tensor(
                    out=d[:], in0=d[:], scalar=1.0, in1=wt[:],
                    op0=Alu.mult, op1=Alu.mult,
                    accum_out=accs[:, i:i + 1])

            for i in range(_NCH):
                if i + 1 < _NCH:
                    at, bt, _, sl = tiles[i + 1]
                    nc.sync.dma_start(out=at[:], in_=a[:, sl])
                    nc.scalar.dma_start(out=bt[:], in_=b[:, sl])
                    load_w(i + 1)
                emit_sub(i)
                if i > 0:
                    emit_stt(i - 1)
            emit_stt(_NCH - 1)

            nc.sync.dma_start(out=out[:], in_=accs[:])

    nc.compile()
    return nc


def _get_nc():
    if "nc" not in _CACHE:
        _CACHE["nc"] = build_nc()
    return _CACHE["nc"]


def make_in_maps(confidence, confidence_gt, weight, depth_and_rotation,
                 ann_values, ann_flags):
    a = np.ascontiguousarray(confidence, dtype=np.float16).reshape(
        _NCORES, 128, _F)
    b = np.ascontiguousarray(confidence_gt, dtype=np.float16).reshape(
        _NCORES, 128, _F)
    import ml_dtypes
    w = np.ascontiguousarray(weight, dtype=np.float32).reshape(
        _NCORES, 128, _F).astype(ml_dtypes.float8_e4m3)
    dr = np.ascontiguousarray(depth_and_rotation, dtype=np.float32).reshape(
        _NCORES, 128, _R * 5)
    an = np.ascontiguousarray(ann_values, dtype=np.float32).reshape(
        _NCORES, 128, _R * 5)
    mk = np.ascontiguousarray(ann_flags).astype(np.float32).reshape(
        _NCORES, 128, _R)
    rio = np.concatenate([dr, an, mk], axis=2)     # [cores, 128, R*11]
    return [dict(a=a[c], b=b[c], w=w[c], rio=rio[c])
            for c in range(_NCORES)]


def reduce_outs(outs):
    """outs: list of per-core {'out': [128, _OUTC]} -> (conf, depth, rot)."""
    P = np.stack([o["out"] for o in outs]).astype(np.float64)
    conf = P[:, :, :_NCH].sum() / float(_HW)
    dep = P[:, :, _NCH].sum() / float(_N)
    rot = P[:, :, _NCH + 1].sum() / float(_N)
    return (np.float32(conf), np.float32(dep), np.float32(rot))


def kernel(confidence, confidence_gt, weight, depth_and_rotation,
           ann_values, ann_flags):
    from concourse.bass_utils import run_bass_kernel_spmd
    nc = _get_nc()
    in_maps = make_in_maps(confidence, confidence_gt, weight,
                           depth_and_rotation, ann_values, ann_flags)
    res = run_bass_kernel_spmd(nc, in_maps, core_ids=list(range(_NCORES)))
    return reduce_outs(res.results)
